# revision 10
# baseline (speedup 1.0000x reference)
"""Trainium2 Bass kernel for MultiHeadAttention (B=4, S=2048, D=1024, H=16, causal).

Sharding: 8 cores = data-parallel over B (4) x tensor-parallel over heads (2 groups
of 8). Core c handles batch c//2, head group c%2. Per-core dataflow (bf16 matmul
operands with fp32 PSUM accumulation, transposed layouts so no on-chip transposes):

  Qt = (wq_g @ x_q.T + bq_g)      [512, S]   (bias added on eviction, per-partition)
  Kt = (wk_g @ x_k.T + bk_g)      [512, S]
  V  = x_v @ wv_g.T               [S, 8*65]  (ones column per head; bv folded into bo_eff)
  per head h, query chunk c (512), key tile kt (128), causal:
     E.T[kt] = Kt_h[:,kt].T @ Qt_h[:,c]      [128, 512]
     P.T = exp(0.125 * E.T)                   (ACT, PSUM->SBUF bf16)
     P.T *= mask01 on diagonal-band tiles     (DVE bf16)
     O_aug.T += V_aug[kt].T @ P.T            [65, 512]  (row 64 = softmax denom s)
     xh = O.T * bcast(1/s)   (bcast via K=1 f32r matmul, reciprocal_approx_fast)
  AllGather(xh over the pair, bf16)  -> x.T full [1024, S]  (per chunk, overlapped)
  out = x.T.T @ wo.T + (bo + wo @ bv)        [S, 1024]  (bias tile added on eviction)

Output: host takes the even core of each pair.
"""

import functools
import sys

import numpy as np

sys.path.insert(0, "/opt/trn_rl_repo")

# --- problem constants (hardcoded; kernel.py must be self-contained) ---
B, S, D, H, HD = 4, 2048, 1024, 16, 64
NCORES = 8
HPC = 8            # heads per core
FLOC = HPC * HD    # 512 local features per core
QCH = 512          # query chunk
KT = 128           # key tile
VW = HD + 1        # V columns per head incl. ones column (65)
NFT = FLOC // 128  # f-tiles per core (4)


def build_program(nc, tile, bass, mybir, seq=S):
    """Emit the per-core SPMD program into `nc` (a Bacc) under a TileContext."""
    dt = mybir.dt
    f32 = dt.float32
    f32r = dt.float32r
    bf16 = dt.bfloat16
    AF = mybir.ActivationFunctionType
    ALU = mybir.AluOpType

    n_tch = seq // QCH          # token chunks (projection + fc + query chunks)
    n_ttile = seq // KT         # 128-token tiles

    # ---- I/O ----
    xqT = nc.dram_tensor("xqT", [D, seq], bf16, kind="ExternalInput").ap()
    xkT = nc.dram_tensor("xkT", [D, seq], bf16, kind="ExternalInput").ap()
    xvT = nc.dram_tensor("xvT", [D, seq], bf16, kind="ExternalInput").ap()
    wqT = nc.dram_tensor("wqT", [D, FLOC], bf16, kind="ExternalInput").ap()
    wkT = nc.dram_tensor("wkT", [D, FLOC], bf16, kind="ExternalInput").ap()
    wvT = nc.dram_tensor("wvT", [D, FLOC], bf16, kind="ExternalInput").ap()
    woT = nc.dram_tensor("woT", [D, FLOC], bf16, kind="ExternalInput").ap()
    bqc = nc.dram_tensor("bqc", [128, NFT], f32, kind="ExternalInput").ap()
    bkc = nc.dram_tensor("bkc", [128, NFT], f32, kind="ExternalInput").ap()
    bor = nc.dram_tensor("bor", [1, FLOC], f32, kind="ExternalInput").ap()
    maskin = nc.dram_tensor("maskin", [KT, KT], bf16, kind="ExternalInput").ap()
    vein = nc.dram_tensor("vein", [KT, HPC], bf16, kind="ExternalInput").ap()
    out = nc.dram_tensor("out", [seq, FLOC], f32, kind="ExternalOutput").ap()

    with tile.TileContext(nc) as tc:
        import contextlib
        ctx = contextlib.ExitStack()
        with ctx:
            # ---------------- pools ----------------
            const = ctx.enter_context(tc.tile_pool(name="const", bufs=1))
            psum = ctx.enter_context(tc.tile_pool(name="psum", bufs=3, space="PSUM"))
            dram = ctx.enter_context(tc.tile_pool(name="dram", bufs=1, space="DRAM"))

            # ---------------- constants ----------------
            ones_f = const.tile([1, QCH], f32)
            nc.vector.memset(ones_f[:], 1.0)
            ones = const.tile([1, QCH], f32r)
            nc.vector.tensor_copy(ones[:], ones_f[:])
            sel1_f = const.tile([128, HD], f32)
            nc.vector.memset(sel1_f[64:66, :], 1.0)
            sel1 = const.tile([128, HD], f32r)   # rows 64..65 = 1.0 (bcast lhsT)
            nc.vector.tensor_copy(sel1[64:66, :], sel1_f[64:66, :])
            mask_sb = const.tile([KT, KT], bf16)   # 0/1 diagonal-block mask
            nc.sync.dma_start(mask_sb[:], maskin[:])
            bq_sb = const.tile([128, NFT], f32)
            nc.sync.dma_start(bq_sb[:], bqc[:])
            bk_sb = const.tile([128, NFT], f32)
            nc.sync.dma_start(bk_sb[:], bkc[:])
            bo_sb = const.tile([1, FLOC], f32r)
            nc.sync.dma_start(bo_sb[:], bor[:].bitcast(f32r))
            ve_sb = const.tile([KT, HPC], bf16)
            nc.sync.dma_start(ve_sb[:], vein[:])

            # PE warm-up: keep TensorE busy while the first DMAs land so the
            # HAM clock-gate opens before real work starts.
            warm_w = const.tile([128, 128], bf16)
            nc.vector.memset(warm_w[:], 0.0)
            warm_x = const.tile([128, QCH], bf16)
            nc.vector.memset(warm_x[:], 0.0)
            for wi in range(80):
                wp = psum.tile([128, QCH], f32, tag="mm", name=f"warm{wi}")
                nc.tensor.matmul(wp[:], lhsT=warm_w[:], rhs=warm_x[:],
                                 start=True, stop=True)

            # fc bias broadcast tile [128, FLOC] = ones.T @ bo_eff_half (built once)
            bo_bc = const.tile([128, FLOC], f32)
            bp = psum.tile([128, QCH], f32, tag="mm", name="bobc")
            nc.tensor.matmul(bp[:], lhsT=ones[0:1, 0:128], rhs=bo_sb[0:1, :],
                             start=True, stop=True)
            nc.vector.tensor_copy(bo_bc[:], bp[:])

            # persistent projection outputs
            qkv = ctx.enter_context(tc.tile_pool(name="qkv", bufs=1))
            qt_tiles = [qkv.tile([128, seq], bf16, tag=f"qt{i}", name=f"qt{i}")
                        for i in range(NFT)]
            kt_tiles = [qkv.tile([128, seq], bf16, tag=f"kt{i}", name=f"kt{i}")
                        for i in range(NFT)]
            v_tiles = [qkv.tile([KT, HPC * VW], bf16, tag=f"v{i}", name=f"v{i}")
                       for i in range(n_ttile)]
            # ones columns of V (col 64 of each head slot)
            for i in range(n_ttile):
                vv = v_tiles[i].rearrange("p (h e) -> p h e", e=VW)
                nc.sync.dma_start(vv[:, :, HD:VW], ve_sb.unsqueeze(2)[:, :, 0:1])

            # ---------------- projections ----------------
            with tc.tile_pool(name="wpool", bufs=8) as wpool, \
                 tc.tile_pool(name="xpool", bufs=16) as xpool:

                def load_w(wsrc):
                    tiles = []
                    for kk in range(D // 128):
                        wt = wpool.tile([128, FLOC], bf16, tag="w", name=f"w{kk}")
                        nc.sync.dma_start(wt[:], wsrc[kk * 128:(kk + 1) * 128, :])
                        tiles.append(wt)
                    return tiles

                def load_x(xsrc, t):
                    tiles = []
                    for kk in range(D // 128):
                        xt = xpool.tile([128, QCH], bf16, tag="x", name=f"x{kk}_{t}")
                        nc.sync.dma_start(
                            xt[:],
                            xsrc[kk * 128:(kk + 1) * 128, t * QCH:(t + 1) * QCH])
                        tiles.append(xt)
                    return tiles

                # Q and K projections (transposed outputs; bias on eviction)
                for name, xsrc, wsrc, bias_sb, dst in (
                    ("q", xqT, wqT, bq_sb, qt_tiles),
                    ("k", xkT, wkT, bk_sb, kt_tiles),
                ):
                    wts = load_w(wsrc)
                    for t in range(n_tch):
                        xts = load_x(xsrc, t)
                        for f in range(NFT):
                            pp = psum.tile([128, QCH], f32, tag="mm",
                                           name=f"pp{name}{t}{f}")
                            for kk in range(D // 128):
                                nc.tensor.matmul(
                                    pp[:],
                                    lhsT=wts[kk][:, f * 128:(f + 1) * 128],
                                    rhs=xts[kk][:],
                                    start=(kk == 0), stop=(kk == D // 128 - 1))
                            nc.vector.tensor_scalar_add(
                                dst[f][:, t * QCH:(t + 1) * QCH], pp[:],
                                bias_sb[:, f:f + 1])
                        del xts

                # V projection (natural layout, no bias)
                wts = load_w(wvT)
                for t in range(n_tch):
                    xts = load_x(xvT, t)
                    for tt in range(QCH // KT):
                        g = t * (QCH // KT) + tt   # global token tile
                        pp = psum.tile([128, FLOC], f32, tag="mm", name=f"ppv{g}")
                        for kk in range(D // 128):
                            nc.tensor.matmul(
                                pp[:],
                                lhsT=xts[kk][:, tt * KT:(tt + 1) * KT],
                                rhs=wts[kk][:],
                                start=(kk == 0), stop=(kk == D // 128 - 1))
                        vv = v_tiles[g].rearrange("p (h e) -> p h e", e=VW)
                        nc.vector.tensor_copy(
                            vv[:, :, 0:HD], pp[:].rearrange("p (h d) -> p h d", d=HD))

            # fc weights (loads overlap attention; placed after proj pools close)
            wo_pool = ctx.enter_context(tc.tile_pool(name="wo", bufs=1))
            wo_sb = []
            for kk in range(D // 128):
                wt = wo_pool.tile([128, FLOC], bf16, tag=f"wo{kk}", name=f"wo{kk}")
                nc.sync.dma_start(wt[:], woT[kk * 128:(kk + 1) * 128, :])
                wo_sb.append(wt)

            # DRAM bounce buffers for the per-chunk AllGather (bf16)
            ag_in = [dram.tile([FLOC, QCH], bf16, tag=f"agi{c}", name=f"agi{c}")
                     for c in range(n_tch)]
            ag_out = [dram.tile([2 * FLOC, QCH], bf16, tag=f"ago{c}", name=f"ago{c}")
                      for c in range(n_tch)]

            # ---------------- attention ----------------
            # Heads are processed in pairs (h0 even on PE rows 0-63, h1 odd on
            # rows 64-127): their K=64 energy matmuls land in disjoint PE
            # row-groups and execute concurrently (~2x on the energy phase).
            with tc.tile_pool(name="pt", bufs=4) as ptpool, \
                 tc.tile_pool(name="att", bufs=4) as attpool:
                for c in range(n_tch):
                    for hp in range(HPC // 2):
                        ft = hp                      # Qt/Kt tile holding this pair
                        heads = (2 * hp, 2 * hp + 1)
                        pvs, eps, pts = {}, {}, {}
                        for h in heads:
                            pvs[h] = psum.tile([VW, QCH], f32, tag=f"pv{h % 2}",
                                               name=f"pv{c}_{h}", bufs=1)
                        nkt = (QCH // KT) * (c + 1)     # causal key tiles
                        for grp in range(nkt // 2):
                            for h in heads:
                                eps[h] = psum.tile([128, 2 * QCH], f32, tag="mm",
                                                   name=f"ep{c}_{h}_{grp}")
                            # kt-interleaved: adjacent matmuls hit disjoint PE
                            # row-groups (h even rows 0-63, h odd rows 64-127)
                            for j2 in range(2):
                                kt = grp * 2 + j2
                                band = kt - (QCH // KT) * c
                                off = band * KT if band > 0 else 0   # causal trim
                                for h in heads:
                                    fr = (h % 2) * HD
                                    nc.tensor.matmul(
                                        eps[h][:, j2 * QCH + off:(j2 + 1) * QCH],
                                        lhsT=kt_tiles[ft][fr:fr + HD,
                                                          kt * KT:(kt + 1) * KT],
                                        rhs=qt_tiles[ft][fr:fr + HD,
                                                         c * QCH + off:(c + 1) * QCH],
                                        start=True, stop=True)
                            for h in heads:
                                pt = ptpool.tile([128, 2 * QCH], bf16,
                                                 tag=f"pt{h % 2}",
                                                 name=f"pt{c}_{h}_{grp}")
                                pts[h] = pt
                                nc.scalar.activation(pt[:], eps[h][:], AF.Exp,
                                                     scale=0.125)
                                for j2 in range(2):
                                    kt = grp * 2 + j2
                                    band = kt - (QCH // KT) * c
                                    if band >= 0:   # mask the diagonal block
                                        sl = pt[:, j2 * QCH + band * KT:
                                                j2 * QCH + (band + 1) * KT]
                                        nc.vector.tensor_tensor(
                                            sl, sl, mask_sb[:], ALU.mult)
                            for j2 in range(2):
                                kt = grp * 2 + j2
                                band = kt - (QCH // KT) * c
                                off = band * KT if band > 0 else 0
                                for h in heads:
                                    nc.tensor.matmul(
                                        pvs[h][:, off:QCH],
                                        lhsT=v_tiles[kt][:, :].rearrange(
                                            "p (h e) -> p h e", e=VW)[:, h, :],
                                        rhs=pts[h][:, j2 * QCH + off:(j2 + 1) * QCH],
                                        start=(kt == 0), stop=(kt == nkt - 1),
                                        skip_group_check=True)
                        # normalize: xh = O.T * bcast(1/s)
                        for h in heads:
                            pv = pvs[h]
                            sr = attpool.tile([128, QCH], f32r, tag="sr",
                                              name=f"sr{c}_{h}")
                            nc.vector.tensor_copy(sr[64:65, :], pv[HD:VW, :])
                            bc = psum.tile([HD, QCH], f32, tag="mm",
                                           name=f"bc{c}_{h}")
                            nc.tensor.matmul(bc[:], lhsT=sel1[64:65, :],
                                             rhs=sr[64:65, :], start=True, stop=True)
                            rcp = attpool.tile([HD, QCH], f32, tag="rcp",
                                               name=f"rcp{c}_{h}")
                            nc.vector.reciprocal_approx_fast(rcp[:], bc[:])
                            xh = attpool.tile([HD, QCH], bf16, tag="xh",
                                              name=f"xh{c}_{h}")
                            nc.vector.tensor_tensor(xh[:], pv[0:HD, :], rcp[:],
                                                    ALU.mult)
                            nc.sync.dma_start(ag_in[c][h * HD:(h + 1) * HD, :],
                                              xh[:])
                    # pairwise AllGather of this chunk's x.T
                    nc.gpsimd.collective_compute(
                        "AllGather", ALU.bypass,
                        replica_groups=[[0, 1], [2, 3], [4, 5], [6, 7]],
                        ins=[ag_in[c].opt()], outs=[ag_out[c].opt()])

            # ---------------- fc_out ----------------
            with tc.tile_pool(name="xf", bufs=16) as xfpool, \
                 tc.tile_pool(name="ost", bufs=3) as ostpool:
                for c in range(n_tch):
                    xf = []
                    for kk in range(D // 128):
                        xt = xfpool.tile([128, QCH], bf16, tag="xf", name=f"xf{c}_{kk}")
                        nc.sync.dma_start(
                            xt[:], ag_out[c][kk * 128:(kk + 1) * 128, :])
                        xf.append(xt)
                    for tt in range(QCH // KT):
                        fp = psum.tile([128, QCH], f32, tag="mm",
                                       name=f"fp{c}_{tt}")
                        for kk in range(D // 128):
                            nc.tensor.matmul(
                                fp[:],
                                lhsT=xf[kk][:, tt * KT:(tt + 1) * KT],
                                rhs=wo_sb[kk][:],
                                start=(kk == 0), stop=(kk == D // 128 - 1))
                        ost = ostpool.tile([128, QCH], f32, tag="ost",
                                           name=f"ost{c}_{tt}")
                        nc.vector.tensor_tensor(ost[:], fp[:], bo_bc[:], ALU.add)
                        nc.sync.dma_start(
                            out[c * QCH + tt * KT:c * QCH + (tt + 1) * KT, :],
                            ost[:])
    return nc


@functools.lru_cache(maxsize=None)
def _compiled(seq=S):
    import concourse.bacc as bacc
    import concourse.bass as bass
    import concourse.mybir as mybir
    import concourse.tile as tile

    nc = bacc.Bacc("TRN2", target_bir_lowering=False, debug=False,
                   num_devices=NCORES)
    build_program(nc, tile, bass, mybir, seq=seq)
    nc.compile()
    return nc


def _host_prep(inputs, seq=S):
    """Build the 8 per-core input maps from full inputs."""
    import ml_dtypes
    bf16 = ml_dtypes.bfloat16

    q, k, v = inputs["query"], inputs["key"], inputs["value"]
    wq, bq = inputs["wq"], inputs["bq"]
    wk, bk = inputs["wk"], inputs["bk"]
    wv, bv = inputs["wv"], inputs["bv"]
    wo, bo = inputs["wo"], inputs["bo"]

    f32 = np.float32
    bo_eff = (bo + wo @ bv).astype(f32)

    # 0/1 diagonal-block mask [128, 128]
    kk = np.arange(KT)[:, None]
    qq = np.arange(KT)[None, :]
    mask = (qq >= kk).astype(bf16)
    ve = np.ones((KT, HPC), bf16)

    in_maps = []
    for core in range(NCORES):
        b, g = core // 2, core % 2
        sl = slice(g * FLOC, (g + 1) * FLOC)
        in_maps.append({
            "xqT": np.ascontiguousarray(q[b, :seq].T).astype(bf16),
            "xkT": np.ascontiguousarray(k[b, :seq].T).astype(bf16),
            "xvT": np.ascontiguousarray(v[b, :seq].T).astype(bf16),
            "wqT": np.ascontiguousarray(wq[sl].T).astype(bf16),
            "wkT": np.ascontiguousarray(wk[sl].T).astype(bf16),
            "wvT": np.ascontiguousarray(wv[sl].T).astype(bf16),
            "woT": np.ascontiguousarray(wo[sl].T).astype(bf16),
            "bqc": np.ascontiguousarray(bq[sl].reshape(NFT, 128).T).astype(f32),
            "bkc": np.ascontiguousarray(bk[sl].reshape(NFT, 128).T).astype(f32),
            "bor": bo_eff[sl].reshape(1, FLOC),
            "maskin": mask,
            "vein": ve,
        })
    return in_maps


def run(inputs, seq=S, trace=False):
    from concourse.bass_utils import run_bass_kernel_spmd

    nc = _compiled(seq)
    in_maps = _host_prep(inputs, seq)
    res = run_bass_kernel_spmd(nc, in_maps, core_ids=list(range(NCORES)),
                               trace=trace)
    out = np.zeros((B, seq, D), np.float32)
    for b in range(B):
        out[b, :, 0:FLOC] = res.results[2 * b]["out"]
        out[b, :, FLOC:D] = res.results[2 * b + 1]["out"]
    return out, res


def kernel(**inputs):
    inputs = {k: np.asarray(v) for k, v in inputs.items()}
    out, _ = run(inputs)
    return out


# revision 11
# speedup vs baseline: 1.0454x; 1.0454x over previous
"""Trainium2 Bass kernel for MultiHeadAttention (B=4, S=2048, D=1024, H=16, causal).

Sharding: 8 cores = data-parallel over B (4) x tensor-parallel over heads (2 groups
of 8). Core c handles batch c//2, head group c%2. Per-core dataflow (bf16 matmul
operands with fp32 PSUM accumulation, transposed layouts so no on-chip transposes):

  Qt = (wq_g @ x_q.T + bq_g)      [512, S]   (bias added on eviction, per-partition)
  Kt = (wk_g @ x_k.T + bk_g)      [512, S]
  V  = x_v @ wv_g.T               [S, 8*65]  (ones column per head; bv folded into bo_eff)
  per head h, query chunk c (512), key tile kt (128), causal:
     E.T[kt] = Kt_h[:,kt].T @ Qt_h[:,c]      [128, 512]   (band tiles causally trimmed)
     P.T = exp(0.125 * E.T)                   (ACT, PSUM->SBUF bf16)
     P.T *= mask01 on the diagonal block      (DVE bf16)
     O_aug.T += V_aug[kt].T @ P.T            [65, 512]  (row 64 = softmax denom s)
     xh = O.T * bcast(1/s)   (bcast via K=1 f32r matmul, reciprocal_approx_fast)
  AllGather(xh over the pair, bf16)  -> x.T full [1024, S]  (per chunk, overlapped)
  out = x.T.T @ wo_half.T + bo_eff_half      [S, 512]   (fc split by output columns)

The whole pipeline is emitted chunk-major (proj -> attention -> AllGather -> fc per
512-token chunk) so the Tile scheduler can fill exp-latency windows on TensorE with
projection/fc matmuls, keeping PE dense (HAM stays warm). Heads run in pairs on
disjoint PE row groups (even rows 0-63, odd 64-127) for concurrent K=64 energies.

Output: host stitches column halves from the core pair of each batch.
"""

import functools
import sys

import numpy as np

sys.path.insert(0, "/opt/trn_rl_repo")

# --- problem constants (hardcoded; kernel.py must be self-contained) ---
B, S, D, H, HD = 4, 2048, 1024, 16, 64
NCORES = 8
HPC = 8            # heads per core
FLOC = HPC * HD    # 512 local features per core
QCH = 512          # query chunk
KT = 128           # key tile
VW = HD + 1        # V columns per head incl. ones column (65)
NFT = FLOC // 128  # f-tiles per core (4)
NKK = D // 128     # contraction k-tiles (8)


def build_program(nc, tile, bass, mybir, seq=S):
    """Emit the per-core SPMD program into `nc` (a Bacc) under a TileContext."""
    dt = mybir.dt
    f32 = dt.float32
    f32r = dt.float32r
    bf16 = dt.bfloat16
    AF = mybir.ActivationFunctionType
    ALU = mybir.AluOpType

    n_tch = seq // QCH          # token chunks
    n_ttile = seq // KT         # 128-token tiles

    # ---- I/O ----
    xqT = nc.dram_tensor("xqT", [D, seq], bf16, kind="ExternalInput").ap()
    xkT = nc.dram_tensor("xkT", [D, seq], bf16, kind="ExternalInput").ap()
    xvT = nc.dram_tensor("xvT", [D, seq], bf16, kind="ExternalInput").ap()
    wqT = nc.dram_tensor("wqT", [D, FLOC], bf16, kind="ExternalInput").ap()
    wkT = nc.dram_tensor("wkT", [D, FLOC], bf16, kind="ExternalInput").ap()
    wvT = nc.dram_tensor("wvT", [D, FLOC], bf16, kind="ExternalInput").ap()
    woT = nc.dram_tensor("woT", [D, FLOC], bf16, kind="ExternalInput").ap()
    bqc = nc.dram_tensor("bqc", [128, NFT], f32, kind="ExternalInput").ap()
    bkc = nc.dram_tensor("bkc", [128, NFT], f32, kind="ExternalInput").ap()
    bor = nc.dram_tensor("bor", [1, FLOC], f32, kind="ExternalInput").ap()
    maskin = nc.dram_tensor("maskin", [KT, KT], bf16, kind="ExternalInput").ap()
    vein = nc.dram_tensor("vein", [KT, HPC], bf16, kind="ExternalInput").ap()
    out = nc.dram_tensor("out", [seq, FLOC], f32, kind="ExternalOutput").ap()

    with tile.TileContext(nc) as tc:
        import contextlib
        ctx = contextlib.ExitStack()
        with ctx:
            # ---------------- pools ----------------
            const = ctx.enter_context(tc.tile_pool(name="const", bufs=1))
            psum = ctx.enter_context(tc.tile_pool(name="psum", bufs=2, space="PSUM"))
            dram = ctx.enter_context(tc.tile_pool(name="dram", bufs=1, space="DRAM"))
            qkv = ctx.enter_context(tc.tile_pool(name="qkv", bufs=1))
            wpool = ctx.enter_context(tc.tile_pool(name="wpool", bufs=1))
            xpool = ctx.enter_context(tc.tile_pool(name="xpool", bufs=32))
            ptpool = ctx.enter_context(tc.tile_pool(name="pt", bufs=4))
            attpool = ctx.enter_context(tc.tile_pool(name="att", bufs=4))
            xfpool = ctx.enter_context(tc.tile_pool(name="xf", bufs=16))
            ostpool = ctx.enter_context(tc.tile_pool(name="ost", bufs=3))

            # ---------------- constants ----------------
            ones_f = const.tile([1, QCH], f32)
            nc.vector.memset(ones_f[:], 1.0)
            ones = const.tile([1, QCH], f32r)
            nc.vector.tensor_copy(ones[:], ones_f[:])
            sel1_f = const.tile([128, HD], f32)
            nc.vector.memset(sel1_f[64:66, :], 1.0)
            sel1 = const.tile([128, HD], f32r)   # rows 64..65 = 1.0 (bcast lhsT)
            nc.vector.tensor_copy(sel1[64:66, :], sel1_f[64:66, :])
            mask_sb = const.tile([KT, KT], bf16)   # 0/1 diagonal-block mask
            nc.sync.dma_start(mask_sb[:], maskin[:])
            bq_sb = const.tile([128, NFT], f32)
            nc.sync.dma_start(bq_sb[:], bqc[:])
            bk_sb = const.tile([128, NFT], f32)
            nc.sync.dma_start(bk_sb[:], bkc[:])
            bo_sb = const.tile([1, FLOC], f32r)
            nc.sync.dma_start(bo_sb[:], bor[:].bitcast(f32r))
            ve_sb = const.tile([KT, HPC], bf16)
            nc.sync.dma_start(ve_sb[:], vein[:])

            # PE warm-up: keep TensorE busy while the first DMAs land so the
            # HAM clock-gate opens before real work starts.
            warm_w = const.tile([128, 128], bf16)
            nc.vector.memset(warm_w[:], 0.0)
            warm_x = const.tile([128, QCH], bf16)
            nc.vector.memset(warm_x[:], 0.0)
            for wi in range(80):
                wp = psum.tile([128, QCH], f32, tag="mm512", name=f"warm{wi}")
                nc.tensor.matmul(wp[:], lhsT=warm_w[:], rhs=warm_x[:],
                                 start=True, stop=True)

            # fc bias broadcast tile [128, FLOC] = ones.T @ bo_eff_half (built once)
            bo_bc = const.tile([128, FLOC], f32)
            bp = psum.tile([128, QCH], f32, tag="mm512", name="bobc")
            nc.tensor.matmul(bp[:], lhsT=ones[0:1, 0:128], rhs=bo_sb[0:1, :],
                             start=True, stop=True)
            nc.vector.tensor_copy(bo_bc[:], bp[:])

            # persistent projection outputs
            qt_tiles = [qkv.tile([128, seq], bf16, tag=f"qt{i}", name=f"qt{i}")
                        for i in range(NFT)]
            kt_tiles = [qkv.tile([128, seq], bf16, tag=f"kt{i}", name=f"kt{i}")
                        for i in range(NFT)]
            v_tiles = [qkv.tile([KT, HPC * VW], bf16, tag=f"v{i}", name=f"v{i}")
                       for i in range(n_ttile)]

            # all projection weights resident (reused every chunk)
            wts = {}
            for wname, wsrc in (("q", wqT), ("k", wkT), ("v", wvT)):
                for kk in range(NKK):
                    wt = wpool.tile([128, FLOC], bf16, tag=f"w{wname}{kk}",
                                    name=f"w{wname}{kk}")
                    nc.sync.dma_start(wt[:], wsrc[kk * 128:(kk + 1) * 128, :])
                    wts[(wname, kk)] = wt
            wo_sb = []
            for kk in range(NKK):
                wt = wpool.tile([128, FLOC], bf16, tag=f"wo{kk}", name=f"wo{kk}")
                nc.sync.dma_start(wt[:], woT[kk * 128:(kk + 1) * 128, :])
                wo_sb.append(wt)

            # DRAM bounce buffers for the per-chunk AllGather (bf16)
            ag_in = [dram.tile([FLOC, QCH], bf16, tag=f"agi{c}", name=f"agi{c}")
                     for c in range(n_tch)]
            ag_out = [dram.tile([2 * FLOC, QCH], bf16, tag=f"ago{c}", name=f"ago{c}")
                      for c in range(n_tch)]

            def load_x(xsrc, pfx, t):
                tiles = []
                for kk in range(NKK):
                    xt = xpool.tile([128, QCH], bf16, tag="x", name=f"x{pfx}{kk}_{t}")
                    nc.sync.dma_start(
                        xt[:], xsrc[kk * 128:(kk + 1) * 128, t * QCH:(t + 1) * QCH])
                    tiles.append(xt)
                return tiles

            def proj_qk(pfx, xts, bias_sb, dst, t):
                for f in range(NFT):
                    pp = psum.tile([128, QCH], f32, tag="mm512",
                                   name=f"pp{pfx}{t}{f}")
                    for kk in range(NKK):
                        nc.tensor.matmul(
                            pp[:], lhsT=wts[(pfx, kk)][:, f * 128:(f + 1) * 128],
                            rhs=xts[kk][:],
                            start=(kk == 0), stop=(kk == NKK - 1))
                    nc.vector.tensor_scalar_add(
                        dst[f][:, t * QCH:(t + 1) * QCH], pp[:], bias_sb[:, f:f + 1])

            def proj_v(xts, t):
                for tt in range(QCH // KT):
                    g = t * (QCH // KT) + tt
                    pp = psum.tile([128, FLOC], f32, tag="mm512", name=f"ppv{g}")
                    for kk in range(NKK):
                        nc.tensor.matmul(
                            pp[:], lhsT=xts[kk][:, tt * KT:(tt + 1) * KT],
                            rhs=wts[("v", kk)][:],
                            start=(kk == 0), stop=(kk == NKK - 1))
                    vv = v_tiles[g].rearrange("p (h e) -> p h e", e=VW)
                    nc.vector.tensor_copy(
                        vv[:, :, 0:HD], pp[:].rearrange("p (h d) -> p h d", d=HD))
                    nc.sync.dma_start(vv[:, :, HD:VW], ve_sb.unsqueeze(2)[:, :, 0:1])

            def attention_pair(c, hp):
                ft = hp
                heads = (2 * hp, 2 * hp + 1)
                pvs, eps, pts = {}, {}, {}
                for h in heads:
                    pvs[h] = psum.tile([VW, QCH], f32, tag=f"pv{h % 2}",
                                       name=f"pv{c}_{h}", bufs=1)
                nkt = (QCH // KT) * (c + 1)     # causal key tiles
                for grp in range(nkt // 2):
                    for h in heads:
                        eps[h] = psum.tile([128, 2 * QCH], f32,
                                           tag=f"epair{h % 2}",
                                           name=f"ep{c}_{h}_{grp}", bufs=1)
                    # kt-interleaved: adjacent matmuls hit disjoint PE row-groups
                    for j2 in range(2):
                        kt = grp * 2 + j2
                        band = kt - (QCH // KT) * c
                        off = band * KT if band > 0 else 0   # causal trim
                        for h in heads:
                            fr = (h % 2) * HD
                            nc.tensor.matmul(
                                eps[h][:, j2 * QCH + off:(j2 + 1) * QCH],
                                lhsT=kt_tiles[ft][fr:fr + HD,
                                                  kt * KT:(kt + 1) * KT],
                                rhs=qt_tiles[ft][fr:fr + HD,
                                                 c * QCH + off:(c + 1) * QCH],
                                start=True, stop=True)
                    for h in heads:
                        pt = ptpool.tile([128, 2 * QCH], bf16, tag=f"pt{h % 2}",
                                         name=f"pt{c}_{h}_{grp}")
                        pts[h] = pt
                        nc.scalar.activation(pt[:], eps[h][:], AF.Exp, scale=0.125)
                        for j2 in range(2):
                            kt = grp * 2 + j2
                            band = kt - (QCH // KT) * c
                            if band >= 0:   # mask the diagonal block
                                sl = pt[:, j2 * QCH + band * KT:
                                        j2 * QCH + (band + 1) * KT]
                                nc.vector.tensor_tensor(sl, sl, mask_sb[:],
                                                        ALU.mult)
                    for j2 in range(2):
                        kt = grp * 2 + j2
                        band = kt - (QCH // KT) * c
                        off = band * KT if band > 0 else 0
                        for h in heads:
                            nc.tensor.matmul(
                                pvs[h][:, off:QCH],
                                lhsT=v_tiles[kt][:, :].rearrange(
                                    "p (h e) -> p h e", e=VW)[:, h, :],
                                rhs=pts[h][:, j2 * QCH + off:(j2 + 1) * QCH],
                                start=(kt == 0), stop=(kt == nkt - 1),
                                skip_group_check=True)
                # normalize: xh = O.T * bcast(1/s)
                for h in heads:
                    pv = pvs[h]
                    sr = attpool.tile([128, QCH], f32r, tag="sr", name=f"sr{c}_{h}")
                    nc.vector.tensor_copy(sr[64:65, :], pv[HD:VW, :])
                    bc = psum.tile([HD, QCH], f32, tag="mm512", name=f"bc{c}_{h}")
                    nc.tensor.matmul(bc[:], lhsT=sel1[64:65, :], rhs=sr[64:65, :],
                                     start=True, stop=True)
                    rcp = attpool.tile([HD, QCH], f32, tag="rcp", name=f"rcp{c}_{h}")
                    nc.vector.reciprocal_approx_fast(rcp[:], bc[:])
                    xh = attpool.tile([HD, QCH], bf16, tag="xh", name=f"xh{c}_{h}")
                    nc.vector.tensor_tensor(xh[:], pv[0:HD, :], rcp[:], ALU.mult)
                    nc.sync.dma_start(ag_in[c][h * HD:(h + 1) * HD, :], xh[:])

            def fc_chunk(c):
                xf = []
                for kk in range(NKK):
                    xt = xfpool.tile([128, QCH], bf16, tag="xf", name=f"xf{c}_{kk}")
                    nc.sync.dma_start(xt[:], ag_out[c][kk * 128:(kk + 1) * 128, :])
                    xf.append(xt)
                for tt in range(QCH // KT):
                    fp = psum.tile([128, QCH], f32, tag="mm512", name=f"fp{c}_{tt}")
                    for kk in range(NKK):
                        nc.tensor.matmul(
                            fp[:], lhsT=xf[kk][:, tt * KT:(tt + 1) * KT],
                            rhs=wo_sb[kk][:],
                            start=(kk == 0), stop=(kk == NKK - 1))
                    ost = ostpool.tile([128, QCH], f32, tag="ost", name=f"ost{c}_{tt}")
                    nc.vector.tensor_tensor(ost[:], fp[:], bo_bc[:], ALU.add)
                    nc.sync.dma_start(
                        out[c * QCH + tt * KT:c * QCH + (tt + 1) * KT, :], ost[:])

            # ---------------- chunk-major pipeline ----------------
            for t in range(n_tch):
                xq = load_x(xqT, "q", t)
                xk = load_x(xkT, "k", t)
                xv = load_x(xvT, "v", t)
                proj_qk("q", xq, bq_sb, qt_tiles, t)
                proj_qk("k", xk, bk_sb, kt_tiles, t)
                proj_v(xv, t)
                for hp in range(HPC // 2):
                    attention_pair(t, hp)
                nc.gpsimd.collective_compute(
                    "AllGather", ALU.bypass,
                    replica_groups=[[0, 1], [2, 3], [4, 5], [6, 7]],
                    ins=[ag_in[t].opt()], outs=[ag_out[t].opt()])
                fc_chunk(t)
    return nc


@functools.lru_cache(maxsize=None)
def _compiled(seq=S):
    import concourse.bacc as bacc
    import concourse.bass as bass
    import concourse.mybir as mybir
    import concourse.tile as tile

    nc = bacc.Bacc("TRN2", target_bir_lowering=False, debug=False,
                   num_devices=NCORES)
    build_program(nc, tile, bass, mybir, seq=seq)
    nc.compile()
    return nc


def _host_prep(inputs, seq=S):
    """Build the 8 per-core input maps from full inputs."""
    import ml_dtypes
    bf16 = ml_dtypes.bfloat16

    q, k, v = inputs["query"], inputs["key"], inputs["value"]
    wq, bq = inputs["wq"], inputs["bq"]
    wk, bk = inputs["wk"], inputs["bk"]
    wv, bv = inputs["wv"], inputs["bv"]
    wo, bo = inputs["wo"], inputs["bo"]

    f32 = np.float32
    bo_eff = (bo + wo @ bv).astype(f32)

    # 0/1 diagonal-block mask [128, 128]
    kk = np.arange(KT)[:, None]
    qq = np.arange(KT)[None, :]
    mask = (qq >= kk).astype(bf16)
    ve = np.ones((KT, HPC), bf16)

    in_maps = []
    for core in range(NCORES):
        b, g = core // 2, core % 2
        sl = slice(g * FLOC, (g + 1) * FLOC)
        in_maps.append({
            "xqT": np.ascontiguousarray(q[b, :seq].T).astype(bf16),
            "xkT": np.ascontiguousarray(k[b, :seq].T).astype(bf16),
            "xvT": np.ascontiguousarray(v[b, :seq].T).astype(bf16),
            "wqT": np.ascontiguousarray(wq[sl].T).astype(bf16),
            "wkT": np.ascontiguousarray(wk[sl].T).astype(bf16),
            "wvT": np.ascontiguousarray(wv[sl].T).astype(bf16),
            "woT": np.ascontiguousarray(wo[sl].T).astype(bf16),
            "bqc": np.ascontiguousarray(bq[sl].reshape(NFT, 128).T).astype(f32),
            "bkc": np.ascontiguousarray(bk[sl].reshape(NFT, 128).T).astype(f32),
            "bor": bo_eff[sl].reshape(1, FLOC),
            "maskin": mask,
            "vein": ve,
        })
    return in_maps


def run(inputs, seq=S, trace=False):
    from concourse.bass_utils import run_bass_kernel_spmd

    nc = _compiled(seq)
    in_maps = _host_prep(inputs, seq)
    res = run_bass_kernel_spmd(nc, in_maps, core_ids=list(range(NCORES)),
                               trace=trace)
    out = np.zeros((B, seq, D), np.float32)
    for b in range(B):
        out[b, :, 0:FLOC] = res.results[2 * b]["out"]
        out[b, :, FLOC:D] = res.results[2 * b + 1]["out"]
    return out, res


def kernel(**inputs):
    inputs = {k: np.asarray(v) for k, v in inputs.items()}
    out, _ = run(inputs)
    return out
